# revision 1
# baseline (speedup 1.0000x reference)
"""Causal self-attention (B=2, T=2048, C=1024, H=16, D=64) on 8 TRN2 NeuronCores.

Tensor-parallel over heads: each core owns 2 heads. w_qkv columns and w_out
rows are sharded by head; x (transposed on host) is replicated. Each core
computes qkv projection -> causal attention -> partial output projection for
its heads; the host sums the 8 partials (the TP all-reduce) and adds b_out.

Device layouts (per core):
  xT      [C=1024, BT=4096]   x transposed (fp16), replicated
  wqkv    [1024, 384]         w_qkv cols  [q_h0 q_h1 k_h0 k_h1 v_h0 v_h1]*64
  bqkv    [128, 3]            matching bias columns per m-tile (fp32)
  wout    [128, 1024]         w_out rows  [h0 h1]*64
  outp    [4096, 1024]        partial output (pre-b_out, fp32)

Matmul operands are fp16 (1 cycle/row, 2-byte weight loads, PSUM accumulates
fp32); scores/softmax stats stay fp32. Scores are computed transposed (keys
on partitions) so softmax needs no transposes: denominators come free from a
ones-column appended to V, and the causal mask is an affine_select on the
exp'd tile. Diagonal score tiles only compute/exp the valid column range
(query >= key); fully masked work is skipped. Compute engines preserve
partition index, so per-head tensors (attnT0/attnT1, wout0/wout1) live at
base partition 0 and partition moves go through DMA (gpsimd) or PE.
Normalization (reciprocal of denominators + scale) is hoisted right after
each (batch, query-block) so the output projection phase starts unblocked.
"""

import numpy as np

import concourse.bass as bass
from concourse import bacc
import concourse.bass_utils as bass_utils
import concourse.mybir as mybir
from concourse.masks import make_identity
from concourse.tile import TileContext

B, T, C, H, D = 2, 2048, 1024, 16, 64
BT = B * T
NCORES = 8
HPC = H // NCORES          # heads per core
JL = 3 * HPC * D           # 384 local qkv output columns
CL = HPC * D               # 128 local channels into out-proj
KT = 128                   # keys per tile (partition dim of scores^T)
QB = 512                   # queries per block (free dim of scores^T)
NQB = T // QB
F32 = mybir.dt.float32
F16 = mybir.dt.float16
AF = mybir.ActivationFunctionType

_cache = {}


def _build_bass():
    nc = bacc.Bacc("TRN2", target_bir_lowering=False, debug=False)
    xT = nc.dram_tensor("xT", [C, BT], F16, kind="ExternalInput").ap()
    wqkv = nc.dram_tensor("wqkv", [C, JL], F16, kind="ExternalInput").ap()
    bqkv = nc.dram_tensor("bqkv", [128, 3], F32, kind="ExternalInput").ap()
    wout = nc.dram_tensor("wout", [CL, C], F16, kind="ExternalInput").ap()
    outp = nc.dram_tensor("outp", [BT, C], F32, kind="ExternalOutput").ap()

    with TileContext(nc) as tc:
        with (
            tc.tile_pool(name="const", bufs=1) as const,
            tc.tile_pool(name="xtp", bufs=3) as xtp,
            tc.tile_pool(name="vtp", bufs=3) as vtp,
            tc.tile_pool(name="ptp", bufs=6) as ptp,
            tc.tile_pool(name="stg", bufs=3) as stg,
            tc.tile_pool(name="rbp", bufs=3) as rbp,
            tc.tile_pool(name="obp", bufs=4) as obp,
            tc.tile_pool(name="psBig", bufs=2, space="PSUM") as psBig,
            tc.tile_pool(name="psT", bufs=1, space="PSUM") as psT,
            tc.tile_pool(name="psPV", bufs=3, space="PSUM") as psPV,
        ):
            # ---- static tensors
            w_sb = const.tile([128, 8, JL], F16)
            nc.sync.dma_start(out=w_sb, in_=wqkv.rearrange("(k p) j -> p k j", p=128))
            wout_sb = const.tile([128, C], F16)
            nc.sync.dma_start(out=wout_sb, in_=wout)
            bias_sb = const.tile([128, 3], F32)
            nc.sync.dma_start(out=bias_sb, in_=bqkv)
            ident = const.tile([128, 128], F16)
            make_identity(nc, ident)
            qT = const.tile([128, BT], F16)    # rows: [h0 d64 | h1 d64]
            kTt = const.tile([128, BT], F16)
            # V in [t, d] tiles + ones column for softmax denominators
            v_sb = const.tile([128, HPC, B, T // KT, D + 1], F16)
            for h in range(HPC):
                for b_ in range(B):
                    nc.vector.memset(v_sb[:, h, b_, :, D:D + 1], 1.0)
            attnTc = const.tile([128, BT], F16)
            attnT1 = const.tile([64, BT], F16)

            # ---- phase A: qkv projection (qkv^T layout) + V transpose
            for tb in range(BT // QB):
                xt = xtp.tile([128, 8, QB], F16, tag="xt", name="xt")
                nc.sync.dma_start(
                    out=xt,
                    in_=xT[:, tb * QB:(tb + 1) * QB].rearrange(
                        "(k p) t -> p k t", p=128))
                for m in range(3):
                    ps = psBig.tile([128, QB], F32, tag="s", name="psp")
                    for k in range(8):
                        nc.tensor.matmul(
                            ps,
                            lhsT=w_sb[:, k, m * 128:(m + 1) * 128],
                            rhs=xt[:, k, :],
                            start=(k == 0), stop=(k == 7))
                    if m == 0:
                        nc.scalar.activation(
                            out=qT[:, tb * QB:(tb + 1) * QB], in_=ps,
                            func=AF.Identity, bias=bias_sb[:, 0:1])
                    elif m == 1:
                        nc.scalar.activation(
                            out=kTt[:, tb * QB:(tb + 1) * QB], in_=ps,
                            func=AF.Identity, bias=bias_sb[:, 1:2])
                    else:
                        vt = vtp.tile([128, QB], F16, tag="vt", name="vt")
                        nc.scalar.activation(
                            out=vt, in_=ps, func=AF.Identity, bias=bias_sb[:, 2:3])
                        for c4 in range(QB // 128):
                            t0 = tb * QB + c4 * 128
                            b_, kt = t0 // T, (t0 % T) // KT
                            for h in range(HPC):
                                pst = psT.tile([128, D], F16, tag="pst", name="pst")
                                nc.tensor.transpose(
                                    pst,
                                    vt[h * 64:(h + 1) * 64, c4 * 128:(c4 + 1) * 128],
                                    ident[h * 64:(h + 1) * 64, h * 64:(h + 1) * 64])
                                nc.vector.tensor_copy(
                                    out=v_sb[:, h, b_, kt, 0:D], in_=pst)

            def attention(b_, h, qb, den_t):
                n_kt = (qb + 1) * (QB // KT)
                pv = psPV.tile([D + 1, QB], F32, tag="pv", name="pv")
                hs = slice(h * 64, (h + 1) * 64)
                q0 = b_ * T + qb * QB
                # full key tiles (kt < 4*qb), processed in pairs
                for kt2 in range(2 * qb):
                    ps = psBig.tile([128, 2 * QB], F32, tag="s", name="pss")
                    for half in range(2):
                        kt = kt2 * 2 + half
                        nc.tensor.matmul(
                            ps[:, half * QB:(half + 1) * QB],
                            lhsT=kTt[hs, b_ * T + kt * KT: b_ * T + (kt + 1) * KT],
                            rhs=qT[hs, q0:q0 + QB], start=True, stop=True)
                    pt = ptp.tile([128, 2 * QB], F16, tag="pt", name="pt")
                    nc.scalar.activation(
                        out=pt, in_=ps, func=AF.Exp, scale=float(D) ** -0.5)
                    for half in range(2):
                        kt = kt2 * 2 + half
                        nc.tensor.matmul(
                            pv,
                            lhsT=v_sb[:, h, b_, kt, :],
                            rhs=pt[:, half * QB:(half + 1) * QB],
                            start=(kt == 0), stop=False)
                # diagonal key tiles: only columns with query >= key are live
                for r in range(QB // KT):
                    kt = qb * (QB // KT) + r
                    off = KT * r
                    w = QB - off
                    ps1 = psBig.tile([128, w], F32, tag="s", name="psd")
                    nc.tensor.matmul(
                        ps1,
                        lhsT=kTt[hs, b_ * T + kt * KT: b_ * T + (kt + 1) * KT],
                        rhs=qT[hs, q0 + off:q0 + QB],
                        start=True, stop=True)
                    pt1 = ptp.tile([128, w], F16, tag="ptd", name="ptd")
                    nc.scalar.activation(
                        out=pt1, in_=ps1, func=AF.Exp, scale=float(D) ** -0.5)
                    # keep exp(score) where local query col >= key row
                    nc.gpsimd.affine_select(
                        out=pt1, in_=pt1,
                        compare_op=mybir.AluOpType.is_ge, fill=0.0,
                        base=0, channel_multiplier=-1, pattern=[[1, w]])
                    nc.tensor.matmul(
                        pv[:, off:QB],
                        lhsT=v_sb[:, h, b_, kt, :],
                        rhs=pt1,
                        start=(kt == 0), stop=(kt == n_kt - 1))
                dst = attnTc[0:D, q0:q0 + QB] if h == 0 else attnT1[:, q0:q0 + QB]
                nc.vector.tensor_copy(out=dst, in_=pv[0:D, :])
                # denominator row: psum p64 -> sbuf p64 -> (dma) den row h
                dstage = stg.tile([D + 1, QB], F32, tag="dstage", name="dstage")
                nc.vector.tensor_copy(out=dstage[D:D + 1, :], in_=pv[D:D + 1, :])
                nc.gpsimd.dma_start(out=den_t[h:h + 1, :], in_=dstage[D:D + 1, :])

            # ---- phase B: attention + per-block normalization
            for b_ in range(B):
                for qb in range(NQB):
                    den_t = stg.tile([HPC, QB], F32, tag="den", name="den")
                    for h in range(HPC):
                        attention(b_, h, qb, den_t)
                    recip_t = stg.tile([HPC, QB], F32, tag="recip", name="recip")
                    rscr_t = stg.tile([HPC, QB], F32, tag="rscr", name="rscr")
                    recip16_t = stg.tile([HPC, QB], F16, tag="recip16", name="recip16")
                    nc.vector.reciprocal_approx_accurate(
                        out=recip_t, in_=den_t, scratch=rscr_t)
                    nc.vector.tensor_copy(out=recip16_t, in_=recip_t)
                    for h in range(HPC):
                        r0 = rbp.tile([1, QB], F16, tag="r0", name="r0")
                        nc.gpsimd.dma_start(out=r0, in_=recip16_t[h:h + 1, :])
                        rb = rbp.tile([D, QB], F16, tag="rb", name="rb")
                        nc.gpsimd.partition_broadcast(rb, r0)
                        cols = slice(b_ * T + qb * QB, b_ * T + (qb + 1) * QB)
                        if h == 0:
                            sl = attnTc[0:D, cols]
                            nc.vector.tensor_mul(sl, sl, rb)
                        else:
                            sl = attnT1[:, cols]
                            nc.vector.tensor_mul(sl, sl, rb)
                            nc.gpsimd.dma_start(
                                out=attnTc[D:2 * D, cols], in_=attnT1[:, cols])

            # ---- phase C: output projection
            for tt in range(BT // 128):
                po = psBig.tile([128, 2 * QB], F32, tag="s", name="po")
                for ch in range(2):
                    nc.tensor.matmul(
                        po[:, ch * QB:(ch + 1) * QB],
                        lhsT=attnTc[:, tt * 128:(tt + 1) * 128],
                        rhs=wout_sb[:, ch * QB:(ch + 1) * QB],
                        start=True, stop=True)
                ob = obp.tile([128, 2 * QB], F32, tag="ob", name="ob")
                nc.any.tensor_copy(out=ob, in_=po)
                nc.sync.dma_start(out=outp[tt * 128:(tt + 1) * 128, :], in_=ob)
    nc.compile()
    return nc


def _prep_in_maps(x, w_qkv, b_qkv, w_out):
    xTfull = np.ascontiguousarray(x.reshape(BT, C).T.astype(np.float16))
    in_maps = []
    for core in range(NCORES):
        hs = [core * HPC + i for i in range(HPC)]
        wq = np.ascontiguousarray(np.concatenate(
            [w_qkv[:, sec * C + h * D: sec * C + (h + 1) * D]
             for sec in range(3) for h in hs], axis=1).astype(np.float16))
        bq = np.ascontiguousarray(np.stack(
            [np.concatenate([b_qkv[sec * C + h * D: sec * C + (h + 1) * D] for h in hs])
             for sec in range(3)], axis=1))
        wo = np.ascontiguousarray(np.concatenate(
            [w_out[h * D:(h + 1) * D, :] for h in hs], axis=0).astype(np.float16))
        in_maps.append({"xT": xTfull, "wqkv": wq, "bqkv": bq, "wout": wo})
    return in_maps


LAST_RESULTS = None


def kernel(x, w_qkv, b_qkv, w_out, b_out):
    global LAST_RESULTS
    x = np.asarray(x, np.float32)
    w_qkv = np.asarray(w_qkv, np.float32)
    b_qkv = np.asarray(b_qkv, np.float32)
    w_out = np.asarray(w_out, np.float32)
    b_out = np.asarray(b_out, np.float32)

    if "nc" not in _cache:
        _cache["nc"] = _build_bass()
    nc = _cache["nc"]

    in_maps = _prep_in_maps(x, w_qkv, b_qkv, w_out)
    res = bass_utils.run_bass_kernel_spmd(nc, in_maps, core_ids=list(range(NCORES)))
    LAST_RESULTS = res

    out = res.results[0]["outp"].copy()
    for r_ in res.results[1:]:
        out += r_["outp"]
    out += b_out
    return out.reshape(B, T, C)



# revision 3
# speedup vs baseline: 1.1961x; 1.1961x over previous
"""Causal self-attention (B=2, T=2048, C=1024, H=16, D=64) on 8 TRN2 NeuronCores.

Tensor-parallel over heads: each core owns 2 heads (w_qkv columns / w_out rows
sharded by head, x replicated). Each core computes qkv -> causal attention ->
partial output projection; the host sums the 8 fp16 partials and adds b_out.

Perf design (vs. the v1 kernel):
  * Scores for the two local heads are computed CONCURRENTLY on the PE array
    via row tiling (K=64 each, tile_position (0,0)/(64,0)) -> 2x score rate.
  * One Exp activation per key-tile round covers both heads ([128,2,512]
    strided AP) -> minimum ACT instruction overhead; ACT does only exp.
  * V^T is produced directly by matmul (x-tile as stationary operand) -- no
    PE transposes; the PSUM->SBUF copy doubles as the v-bias add (DVE).
  * All PSUM->SBUF moves are explicit nc.vector ops (never nc.any -> ACT).
  * Output partials are written fp16 (halves HBM traffic; host sums in fp32).
  * Emission is software-pipelined per 512-token block: qkv(tb) ->
    attention(tb) with out-proj(tb-1) matmuls interleaved between rounds ->
    normalize(tb).  This keeps the PE stream dense so the HAM clock gate
    stays at full rate (the v1 kernel ran phases B/C at half clock).
  * PSUM budget (8 banks): scores 2x[128,1024] (4) + pv 2x[65,512] (2)
    + proj [128,512] (1) + out-proj [128,512] (1).
"""

import numpy as np

import concourse.bass as bass
from concourse import bacc
import concourse.bass_utils as bass_utils
import concourse.mybir as mybir
from concourse.tile import TileContext

B, T, C, H, D = 2, 2048, 1024, 16, 64
BT = B * T
NCORES = 8
HPC = H // NCORES          # heads per core = 2
JL = 3 * HPC * D           # 384 local qkv output columns
CL = HPC * D               # 128 local channels into out-proj
KT = 128                   # keys per tile (partition dim of scores^T)
QB = 512                   # queries per block (free dim of scores^T)
NQB = T // QB              # 4 query blocks per batch
NTB = BT // QB             # 8 token blocks total
F32 = mybir.dt.float32
F16 = mybir.dt.float16
AF = mybir.ActivationFunctionType

_cache = {}


def _build_bass():
    nc = bacc.Bacc("TRN2", target_bir_lowering=False, debug=False)
    xT = nc.dram_tensor("xT", [C, BT], F16, kind="ExternalInput").ap()
    wqkv = nc.dram_tensor("wqkv", [C, JL], F16, kind="ExternalInput").ap()
    bqk = nc.dram_tensor("bqk", [128, 2], F32, kind="ExternalInput").ap()
    bv16 = nc.dram_tensor("bv16", [1, 128], F16, kind="ExternalInput").ap()
    wout = nc.dram_tensor("wout", [CL, C], F16, kind="ExternalInput").ap()
    outp = nc.dram_tensor("outp", [BT, C], F16, kind="ExternalOutput").ap()

    with TileContext(nc) as tc:
        with (
            tc.tile_pool(name="const", bufs=1) as const,
            tc.tile_pool(name="xtp", bufs=3) as xtp,
            tc.tile_pool(name="ptp", bufs=3) as ptp,
            tc.tile_pool(name="stg", bufs=2) as stg,
            tc.tile_pool(name="rbp", bufs=2) as rbp,
            tc.tile_pool(name="obp", bufs=3) as obp,
            tc.tile_pool(name="psS", bufs=2, space="PSUM") as psS,
            tc.tile_pool(name="psPV", bufs=1, space="PSUM") as psPV,
            tc.tile_pool(name="psA", bufs=1, space="PSUM") as psA,
            tc.tile_pool(name="psO", bufs=1, space="PSUM") as psO,
        ):
            # ---- static tensors
            w_sb = const.tile([128, 8, JL], F16)
            nc.sync.dma_start(out=w_sb, in_=wqkv.rearrange("(k p) j -> p k j", p=128))
            wout_sb = const.tile([128, C], F16)
            nc.sync.dma_start(out=wout_sb, in_=wout)
            bqk_sb = const.tile([128, 2], F32)
            nc.sync.dma_start(out=bqk_sb, in_=bqk)
            bvrow = const.tile([1, 128], F16)
            nc.sync.dma_start(out=bvrow, in_=bv16)
            vb_bcast = const.tile([128, 128], F16)
            nc.gpsimd.partition_broadcast(vb_bcast, bvrow)
            qT = const.tile([128, BT], F16)    # rows: [h0 d64 | h1 d64]
            kTt = const.tile([128, BT], F16)
            # V in [t, d] tiles + ones column for softmax denominators
            v_sb = const.tile([128, HPC, B, T // KT, D + 1], F16)
            for h in range(HPC):
                for b_ in range(B):
                    nc.vector.memset(v_sb[:, h, b_, :, D:D + 1], 1.0)
            attnTc = const.tile([128, BT], F16)
            attnT1 = const.tile([64, BT], F16)

            xt_t = {}

            def emit_xt_dma(tb):
                xt = xtp.tile([128, 8, QB], F16, tag="xt", name="xt")
                nc.sync.dma_start(
                    out=xt,
                    in_=xT[:, tb * QB:(tb + 1) * QB].rearrange(
                        "(k p) t -> p k t", p=128))
                xt_t[tb] = xt

            def emit_qk(tb):
                xt = xt_t[tb]
                for m in range(2):
                    ps = psA.tile([128, QB], F32, tag="pa", name="psqk")
                    for k in range(8):
                        nc.tensor.matmul(
                            ps,
                            lhsT=w_sb[:, k, m * 128:(m + 1) * 128],
                            rhs=xt[:, k, :],
                            start=(k == 0), stop=(k == 7))
                    dst = (qT if m == 0 else kTt)[:, tb * QB:(tb + 1) * QB]
                    nc.vector.tensor_scalar_add(dst, ps, bqk_sb[:, m:m + 1])

            def emit_vT(tb, c4):
                b_ = tb // NQB
                kt = (tb % NQB) * (QB // KT) + c4
                xt = xt_t[tb]
                ps = psA.tile([128, 128], F32, tag="pa", name="psv")
                for k in range(8):
                    nc.tensor.matmul(
                        ps,
                        lhsT=xt[:, k, c4 * 128:(c4 + 1) * 128],
                        rhs=w_sb[:, k, 256:384],
                        start=(k == 0), stop=(k == 7))
                for h in range(HPC):
                    nc.vector.tensor_add(
                        v_sb[:, h, b_, kt, 0:D],
                        ps[:, h * 64:(h + 1) * 64],
                        vb_bcast[:, h * 64:(h + 1) * 64])

            def emit_round(b_, qb, kt, pv, n_kt):
                """One key-tile round: packed scores -> exp -> (mask) -> PV."""
                q0 = b_ * T + qb * QB
                diag = kt >= qb * (QB // KT)
                off = KT * (kt - qb * (QB // KT)) if diag else 0
                w = QB - off
                ks = slice(b_ * T + kt * KT, b_ * T + (kt + 1) * KT)
                ps = psS.tile([128, 2, QB], F32, tag="ps", name="ps")
                for h in range(HPC):
                    hs = slice(h * 64, (h + 1) * 64)
                    nc.tensor.matmul(
                        ps[:, h, 0:w],
                        lhsT=kTt[hs, ks],
                        rhs=qT[hs, q0 + off:q0 + QB],
                        start=True, stop=True,
                        tile_position=(h * 64, 0))
                pt = ptp.tile([128, 2, QB], F16, tag="pt", name="pt")
                nc.scalar.activation(
                    out=pt[:, :, 0:w], in_=ps[:, :, 0:w],
                    func=AF.Exp, scale=float(D) ** -0.5)
                if diag:
                    # keep exp(score) where local query col >= key row
                    nc.gpsimd.affine_select(
                        out=pt[:, :, 0:w], in_=pt[:, :, 0:w],
                        compare_op=mybir.AluOpType.is_ge, fill=0.0,
                        base=0, channel_multiplier=-1, pattern=[[0, 2], [1, w]])
                for h in range(HPC):
                    nc.tensor.matmul(
                        pv[h][:, off:QB],
                        lhsT=v_sb[:, h, b_, kt, :],
                        rhs=pt[:, h, 0:w],
                        start=(kt == 0), stop=(kt == n_kt - 1))

            def emit_norm(tb, pv):
                b_, qb = tb // NQB, tb % NQB
                cols = slice(b_ * T + qb * QB, b_ * T + (qb + 1) * QB)
                # denominator rows (psum p64) -> sbuf p64 -> (dma) den rows 0,1
                dstage = stg.tile([D + 1, QB], F32, tag="dstage", name="dstage")
                den_t = stg.tile([HPC, QB], F32, tag="den", name="den")
                nc.vector.tensor_copy(out=dstage[D:D + 1, :], in_=pv[0][D:D + 1, :])
                nc.gpsimd.dma_start(out=den_t[0:1, :], in_=dstage[D:D + 1, :])
                dstage2 = stg.tile([D + 1, QB], F32, tag="dstage2", name="dstage2")
                nc.vector.tensor_copy(out=dstage2[D:D + 1, :], in_=pv[1][D:D + 1, :])
                nc.gpsimd.dma_start(out=den_t[1:2, :], in_=dstage2[D:D + 1, :])
                recip_t = stg.tile([HPC, QB], F32, tag="recip", name="recip")
                rscr_t = stg.tile([HPC, QB], F32, tag="rscr", name="rscr")
                recip16_t = stg.tile([HPC, QB], F16, tag="recip16", name="recip16")
                nc.vector.reciprocal_approx_accurate(
                    out=recip_t, in_=den_t, scratch=rscr_t)
                nc.vector.tensor_copy(out=recip16_t, in_=recip_t)
                for h in range(HPC):
                    r0 = rbp.tile([1, QB], F16, tag="r0", name="r0")
                    nc.gpsimd.dma_start(out=r0, in_=recip16_t[h:h + 1, :])
                    rb = rbp.tile([D, QB], F16, tag="rb", name="rb")
                    nc.gpsimd.partition_broadcast(rb, r0)
                    if h == 0:
                        nc.vector.tensor_mul(attnTc[0:D, cols], pv[0][0:D, :], rb)
                    else:
                        nc.vector.tensor_mul(attnT1[:, cols], pv[1][0:D, :], rb)
                        nc.gpsimd.dma_start(
                            out=attnTc[D:2 * D, cols], in_=attnT1[:, cols])

            def emit_oproj(tb, i):
                tt = tb * (QB // 128) + i // 2
                ch = i % 2
                po = psO.tile([128, QB], F32, tag="po", name="po")
                nc.tensor.matmul(
                    po,
                    lhsT=attnTc[:, tt * 128:(tt + 1) * 128],
                    rhs=wout_sb[:, ch * QB:(ch + 1) * QB],
                    start=True, stop=True)
                ob = obp.tile([128, QB], F16, tag="ob", name="ob")
                nc.vector.tensor_copy(out=ob, in_=po)
                nc.sync.dma_start(
                    out=outp[tt * 128:(tt + 1) * 128, ch * QB:(ch + 1) * QB],
                    in_=ob)

            # ---- software-pipelined emission
            emit_xt_dma(0)
            emit_xt_dma(1)
            for tb in range(NTB):
                b_, qb = tb // NQB, tb % NQB
                if tb + 2 < NTB:
                    emit_xt_dma(tb + 2)
                emit_qk(tb)
                for c4 in range(QB // 128):
                    emit_vT(tb, c4)
                n_kt = (qb + 1) * (QB // KT)
                pv = [psPV.tile([D + 1, QB], F32, tag=f"pv{h}", name=f"pv{h}")
                      for h in range(HPC)]
                # interleave out-proj of the previous block between rounds
                fills = [] if tb == 0 else list(range(8))
                for j in range(n_kt):
                    emit_round(b_, qb, j, pv, n_kt)
                    lo = len(fills) * j // n_kt
                    hi = len(fills) * (j + 1) // n_kt
                    for i in fills[lo:hi]:
                        emit_oproj(tb - 1, i)
                emit_norm(tb, pv)
            for i in range(8):
                emit_oproj(NTB - 1, i)
    nc.compile()
    return nc


def _prep_in_maps(x, w_qkv, b_qkv, w_out):
    xTfull = np.ascontiguousarray(x.reshape(BT, C).T.astype(np.float16))
    in_maps = []
    for core in range(NCORES):
        hs = [core * HPC + i for i in range(HPC)]
        wq = np.ascontiguousarray(np.concatenate(
            [w_qkv[:, sec * C + h * D: sec * C + (h + 1) * D]
             for sec in range(3) for h in hs], axis=1).astype(np.float16))
        bqk_ = np.ascontiguousarray(np.stack(
            [np.concatenate([b_qkv[sec * C + h * D: sec * C + (h + 1) * D]
                             for h in hs])
             for sec in range(2)], axis=1).astype(np.float32))
        bv_ = np.ascontiguousarray(np.concatenate(
            [b_qkv[2 * C + h * D: 2 * C + (h + 1) * D] for h in hs]
        ).astype(np.float16).reshape(1, 128))
        wo = np.ascontiguousarray(np.concatenate(
            [w_out[h * D:(h + 1) * D, :] for h in hs], axis=0).astype(np.float16))
        in_maps.append({"xT": xTfull, "wqkv": wq, "bqk": bqk_, "bv16": bv_,
                        "wout": wo})
    return in_maps


LAST_RESULTS = None


def kernel(x, w_qkv, b_qkv, w_out, b_out):
    global LAST_RESULTS
    x = np.asarray(x, np.float32)
    w_qkv = np.asarray(w_qkv, np.float32)
    b_qkv = np.asarray(b_qkv, np.float32)
    w_out = np.asarray(w_out, np.float32)
    b_out = np.asarray(b_out, np.float32)

    if "nc" not in _cache:
        _cache["nc"] = _build_bass()
    nc = _cache["nc"]

    in_maps = _prep_in_maps(x, w_qkv, b_qkv, w_out)
    res = bass_utils.run_bass_kernel_spmd(nc, in_maps, core_ids=list(range(NCORES)))
    LAST_RESULTS = res

    out = res.results[0]["outp"].astype(np.float32)
    for r_ in res.results[1:]:
        out += r_["outp"].astype(np.float32)
    out += b_out
    return out.reshape(B, T, C)
